# revision 5
# baseline (speedup 1.0000x reference)
"""Trainium2 (8 NeuronCores) kernel for a 2D self-attention block.

Reference computation (per image, c=512 channels, t=h*w=1024 tokens, 8 heads):
    qkv  = w_qkv @ x + b_qkv           (1x1 conv == channel matmul)
    q,k,v split; per head: attn = softmax(q^T k / sqrt(64)); o = attn @ v
    out  = w_proj @ o + b_proj

Sharding: pure data-parallel — batch 16 split 2 images/core across 8 cores,
weights broadcast. No collectives needed.

Per-core dataflow (all matmul operands bf16, fp32 PSUM accumulation):
  - host pre-transposes weights -> wT (c-major contraction layouts on device)
  - Q,K computed channel-major (e,t); V computed token-major (t,e) so the
    attention matmuls need no on-chip transposes:
        scoresT = K_h^T Q_h   (T on partitions, t free)
        p = exp(scoresT/8)    (ScalarE, bf16 out; no max-subtraction needed:
                               logits are O(1) for this distribution)
        outT_aug = [V_h | 1]^T-chunks contracted with p  -> (65, t) where row
                   64 accumulates the softmax denominator (ones column trick)
        attn = outT_aug[0:64] * broadcast(1/row64)   (GpSimd partition bcast)
  - v-bias folds into an effective proj bias on host (softmax weights sum to 1)
  - proj: y^T-free layout (o on partitions, t free) -> direct DMA out
"""

import os
import sys
import threading

import numpy as np
import ml_dtypes

_REPO = "/opt/trn_rl_repo"
if _REPO not in sys.path:
    sys.path.insert(0, _REPO)

B, C, T = 16, 512, 1024
NH, E = 8, 64
NCORES = 8
BLOC = B // NCORES            # images per core
CK = C // 128                 # contraction chunks over channels
TK = T // 128                 # chunks over the T (attended) token axis
NT = T // 512                 # 512-wide tiles over the t axis
P = 128
SOFTMAX_SCALE = 1.0 / 8.0     # 1/sqrt(E)

_cache = threading.local()


def _build_nc(reps=1):
    import concourse.tile as tile
    from concourse import bacc, mybir

    F32 = mybir.dt.float32
    BF16 = mybir.dt.bfloat16
    EXP = mybir.ActivationFunctionType.Exp

    nc = bacc.Bacc(None, target_bir_lowering=False, debug=False)
    x_ext = nc.declare_dram_parameter("x", [BLOC, C, T], BF16, isOutput=False)
    wqkv_ext = nc.declare_dram_parameter("wqkvT", [C, 3 * C], BF16, isOutput=False)
    wproj_ext = nc.declare_dram_parameter("wprojT", [C, C], BF16, isOutput=False)
    bqk_ext = nc.declare_dram_parameter("bqk", [P, 8], F32, isOutput=False)
    bproj_ext = nc.declare_dram_parameter("bproj", [P, CK], F32, isOutput=False)
    out_ext = nc.declare_dram_parameter("out", [BLOC, C, T], F32, isOutput=True)

    with tile.TileContext(nc) as tc:
        with (
            tc.tile_pool(name="consts", bufs=1) as consts,
            tc.tile_pool(name="xp", bufs=2) as xp,
            tc.tile_pool(name="qkp", bufs=2) as qkp,
            tc.tile_pool(name="vp", bufs=2) as vp,
            tc.tile_pool(name="pp", bufs=3) as pp,
            tc.tile_pool(name="atp", bufs=2) as atp,
            tc.tile_pool(name="sp", bufs=4) as sp,
            tc.tile_pool(name="qk_ps", bufs=2, space="PSUM") as qk_ps,
            tc.tile_pool(name="av_ps", bufs=2, space="PSUM") as av_ps,
            tc.tile_pool(name="sm_ps", bufs=2, space="PSUM") as sm_ps,
        ):
            wqkv_sb = consts.tile([P, CK, 3 * C], BF16)
            nc.sync.dma_start(
                wqkv_sb[:], wqkv_ext.rearrange("(ck p) o -> p ck o", p=P)
            )
            wproj_sb = consts.tile([P, CK, C], BF16)
            nc.sync.dma_start(
                wproj_sb[:], wproj_ext.rearrange("(ck p) o -> p ck o", p=P)
            )
            bqk_sb = consts.tile([P, 8], F32)
            nc.sync.dma_start(bqk_sb[:], bqk_ext[:])
            bproj_sb = consts.tile([P, CK], F32)
            nc.sync.dma_start(bproj_sb[:], bproj_ext[:])

            for b in [b for _ in range(reps) for b in range(BLOC)]:
                x_t = xp.tile([P, CK, T], BF16)
                nc.sync.dma_start(
                    x_t[:], x_ext[b].rearrange("(ck p) t -> p ck t", p=P)
                )

                # Q, K projections: channel-major (e on partitions, t free)
                q_t = qkp.tile([P, CK, T], BF16, tag="q")
                k_t = qkp.tile([P, CK, T], BF16, tag="k")
                for oc in range(2 * CK):
                    dst = q_t if oc < CK else k_t
                    for nt in range(NT):
                        ps = sm_ps.tile([P, 512], F32, tag="sm")
                        for ck in range(CK):
                            nc.tensor.matmul(
                                ps[:],
                                wqkv_sb[:, ck, oc * 128 : (oc + 1) * 128],
                                x_t[:, ck, nt * 512 : (nt + 1) * 512],
                                start=(ck == 0),
                                stop=(ck == CK - 1),
                            )
                        nc.vector.tensor_scalar_add(
                            dst[:, oc % CK, nt * 512 : (nt + 1) * 512],
                            ps[:],
                            bqk_sb[:, oc : oc + 1],
                        )

                # V^T: token-major (t on partitions, v-channels free), augmented
                # with a ones column per head (65th column -> softmax denom)
                v_t = vp.tile([P, TK, NH * 65], BF16)
                v4 = v_t.rearrange("p tk (h e) -> p tk h e", e=65)
                nc.gpsimd.memset(v4[:, :, :, 64], 1.0)
                for tt in range(TK):
                    ps = sm_ps.tile([P, 512], F32, tag="sm")
                    for ck in range(CK):
                        nc.tensor.matmul(
                            ps[:],
                            x_t[:, ck, tt * 128 : (tt + 1) * 128],
                            wqkv_sb[:, ck, 2 * C : 3 * C],
                            start=(ck == 0),
                            stop=(ck == CK - 1),
                        )
                    nc.vector.tensor_copy(
                        v4[:, tt, :, 0:64],
                        ps.rearrange("p (h e) -> p h e", e=64),
                    )

                # attention per head
                attn_t = atp.tile([P, CK, T], BF16)
                for h in range(NH):
                    ec, eo = h // 2, (h % 2) * 64
                    pT = pp.tile([P, TK * T], BF16, tag="pT")  # flat (tk, t)
                    for tk in range(TK):
                        ps = qk_ps.tile([P, 1024], F32)
                        for nt in range(NT):
                            nc.tensor.matmul(
                                ps[:, nt * 512 : (nt + 1) * 512],
                                k_t[eo : eo + 64, ec, tk * 128 : (tk + 1) * 128],
                                q_t[eo : eo + 64, ec, nt * 512 : (nt + 1) * 512],
                                start=True,
                                stop=True,
                            )
                        nc.scalar.activation(
                            pT[:, tk * T : (tk + 1) * T],
                            ps[:],
                            EXP,
                            scale=SOFTMAX_SCALE,
                        )
                    for nt in range(NT):
                        avp = av_ps.tile([65, 512], F32)
                        for tk in range(TK):
                            nc.tensor.matmul(
                                avp[:],
                                v_t[:, tk, h * 65 : (h + 1) * 65],
                                pT[:, tk * T + nt * 512 : tk * T + (nt + 1) * 512],
                                start=(tk == 0),
                                stop=(tk == TK - 1),
                            )
                        rec = sp.tile([1, 512], F32, tag="rec")
                        nc.vector.reciprocal(rec[:], avp[64:65, :])
                        rb = sp.tile([64, 512], F32, tag="rb")
                        nc.gpsimd.partition_broadcast(rb[:], rec[:])
                        nc.vector.tensor_mul(
                            attn_t[eo : eo + 64, ec, nt * 512 : (nt + 1) * 512],
                            avp[0:64, :],
                            rb[:],
                        )

                # output projection (o on partitions, t free) + bias, DMA out
                for ot in range(CK):
                    for nt in range(NT):
                        ps = sm_ps.tile([P, 512], F32, tag="sm")
                        for ck in range(CK):
                            nc.tensor.matmul(
                                ps[:],
                                wproj_sb[:, ck, ot * 128 : (ot + 1) * 128],
                                attn_t[:, ck, nt * 512 : (nt + 1) * 512],
                                start=(ck == 0),
                                stop=(ck == CK - 1),
                            )
                        y = sp.tile([P, 512], F32, tag="y")
                        nc.vector.tensor_scalar_add(
                            y[:], ps[:], bproj_sb[:, ot : ot + 1]
                        )
                        nc.sync.dma_start(
                            out_ext[
                                b, ot * 128 : (ot + 1) * 128, nt * 512 : (nt + 1) * 512
                            ],
                            y[:],
                        )
    nc.compile()
    return nc


def _get_nc():
    if not hasattr(_cache, "nc"):
        _cache.nc = _build_nc()
    return _cache.nc


def _prepare_in_maps(x, w_qkv, b_qkv, w_proj, b_proj):
    x = np.asarray(x, dtype=np.float32)
    w_qkv = np.asarray(w_qkv, dtype=np.float32)
    b_qkv = np.asarray(b_qkv, dtype=np.float32)
    w_proj = np.asarray(w_proj, dtype=np.float32)
    b_proj = np.asarray(b_proj, dtype=np.float32)

    bf16 = ml_dtypes.bfloat16
    wqkvT = np.ascontiguousarray(w_qkv.T).astype(bf16)          # (C, 3C)
    wprojT = np.ascontiguousarray(w_proj.T).astype(bf16)        # (C, C)
    # per-partition bias layouts: bias[j*128 + p] -> [p, j]
    bqk = np.ascontiguousarray(b_qkv[: 2 * C].reshape(2 * CK, P).T)
    # v-bias folds into the projection bias (softmax weights sum to 1)
    bproj_eff = w_proj @ b_qkv[2 * C :] + b_proj
    bproj = np.ascontiguousarray(bproj_eff.reshape(CK, P).T)

    xs = x.reshape(B, C, T).astype(bf16)
    in_maps = []
    for i in range(NCORES):
        in_maps.append(
            {
                "x": np.ascontiguousarray(xs[i * BLOC : (i + 1) * BLOC]),
                "wqkvT": wqkvT,
                "wprojT": wprojT,
                "bqk": bqk,
                "bproj": bproj,
            }
        )
    return in_maps


def kernel(x, w_qkv, b_qkv, w_proj, b_proj, _trace=False):
    from concourse.bass_utils import run_bass_kernel_spmd

    in_maps = _prepare_in_maps(x, w_qkv, b_qkv, w_proj, b_proj)
    nc = _get_nc()
    res = run_bass_kernel_spmd(
        nc, in_maps, core_ids=list(range(NCORES)), trace=_trace
    )
    out = np.concatenate([r["out"] for r in res.results], axis=0)
    out = out.reshape(B, C, 32, 32)
    if _trace:
        return out, res
    return out


if __name__ == "__main__":
    rng = np.random.default_rng(0)
    ins = {
        "x": rng.standard_normal((B, C, 32, 32), dtype=np.float32),
        "w_qkv": rng.standard_normal((3 * C, C), dtype=np.float32) / np.sqrt(C),
        "b_qkv": np.zeros(3 * C, np.float32),
        "w_proj": rng.standard_normal((C, C), dtype=np.float32) / np.sqrt(C),
        "b_proj": np.zeros(C, np.float32),
    }
    o = kernel(**ins)
    print("out", o.shape, o.dtype, float(np.abs(o).mean()))


# revision 8
# speedup vs baseline: 1.3659x; 1.3659x over previous
"""Trainium2 (8 NeuronCores) kernel for a 2D self-attention block.

Reference computation (per image, c=512 channels, t=h*w=1024 tokens, 8 heads):
    qkv  = w_qkv @ x + b_qkv           (1x1 conv == channel matmul)
    q,k,v split; per head: attn = softmax(q^T k / sqrt(64)); o = attn @ v
    out  = w_proj @ o + b_proj

Sharding: pure data-parallel — batch 16 split 2 images/core across 8 cores,
weights broadcast. No collectives needed.

Per-core dataflow (all matmul operands bf16, fp32 PSUM accumulation):
  - host pre-transposes weights -> wT (c-major contraction layouts on device)
  - Q,K computed channel-major (e,t); V computed token-major (t,e) so the
    attention matmuls need no on-chip transposes:
        scoresT = K_h^T Q_h   (T on partitions, t free)
        p = exp(scoresT/8)    (ScalarE, bf16 out; no max-subtraction needed:
                               logits are O(1) for this distribution)
        outT_aug = [V_h | 1]^T-chunks contracted with p  -> (65, t) where row
                   64 accumulates the softmax denominator (ones column trick)
        attn = outT_aug[0:64] * broadcast(1/row64)   (GpSimd partition bcast)
  - v-bias folds into an effective proj bias on host (softmax weights sum to 1)
  - proj: y^T-free layout (o on partitions, t free) -> direct DMA out
"""

import os
import sys
import threading

import numpy as np
import ml_dtypes

_REPO = "/opt/trn_rl_repo"
if _REPO not in sys.path:
    sys.path.insert(0, _REPO)

B, C, T = 16, 512, 1024
NH, E = 8, 64
NCORES = 8
BLOC = B // NCORES            # images per core
CK = C // 128                 # contraction chunks over channels
TK = T // 128                 # chunks over the T (attended) token axis
NT = T // 512                 # 512-wide tiles over the t axis
P = 128
SOFTMAX_SCALE = 1.0 / 8.0     # 1/sqrt(E)

_cache = threading.local()


def _build_nc(reps=1, mode="full"):
    import concourse.tile as tile
    from concourse import bacc, mybir

    F32 = mybir.dt.float32
    BF16 = mybir.dt.bfloat16
    EXP = mybir.ActivationFunctionType.Exp

    nc = bacc.Bacc(None, target_bir_lowering=False, debug=False)
    x_ext = nc.declare_dram_parameter("x", [BLOC, C, T], BF16, isOutput=False)
    wqkv_ext = nc.declare_dram_parameter("wqkvT", [C, 3 * C], BF16, isOutput=False)
    wproj_ext = nc.declare_dram_parameter("wprojT", [C, C], BF16, isOutput=False)
    bqk_ext = nc.declare_dram_parameter("bqk", [P, 8], F32, isOutput=False)
    bproj_ext = nc.declare_dram_parameter("bproj", [P, CK], F32, isOutput=False)
    out_ext = nc.declare_dram_parameter("out", [BLOC, C, T], F32, isOutput=True)

    with tile.TileContext(nc) as tc:
        with (
            tc.tile_pool(name="consts", bufs=1) as consts,
            tc.tile_pool(name="xp", bufs=2) as xp,
            tc.tile_pool(name="qkp", bufs=2) as qkp,
            tc.tile_pool(name="vp", bufs=2) as vp,
            tc.tile_pool(name="pp", bufs=3) as pp,
            tc.tile_pool(name="atp", bufs=2) as atp,
            tc.tile_pool(name="sp", bufs=4) as sp,
            tc.tile_pool(name="qk_ps", bufs=2, space="PSUM") as qk_ps,
            tc.tile_pool(name="av_ps", bufs=2, space="PSUM") as av_ps,
            tc.tile_pool(name="sm_ps", bufs=2, space="PSUM") as sm_ps,
        ):
            wqkv_sb = consts.tile([P, CK, 3 * C], BF16)
            nc.sync.dma_start(
                wqkv_sb[:], wqkv_ext.rearrange("(ck p) o -> p ck o", p=P)
            )
            wproj_sb = consts.tile([P, CK, C], BF16)
            nc.sync.dma_start(
                wproj_sb[:], wproj_ext.rearrange("(ck p) o -> p ck o", p=P)
            )
            bqk_sb = consts.tile([P, 8], F32)
            nc.sync.dma_start(bqk_sb[:], bqk_ext[:])
            bproj_sb = consts.tile([P, CK], F32)
            nc.sync.dma_start(bproj_sb[:], bproj_ext[:])

            for b in [b for _ in range(reps) for b in range(BLOC)]:
                x_t = xp.tile([P, CK, T], BF16)
                nc.sync.dma_start(
                    x_t[:], x_ext[b].rearrange("(ck p) t -> p ck t", p=P)
                )

                # Q, K projections: channel-major (e on partitions, t free)
                q_t = qkp.tile([P, CK, T], BF16, tag="q")
                k_t = qkp.tile([P, CK, T], BF16, tag="k")
                for oc in range(2 * CK):
                    dst = q_t if oc < CK else k_t
                    for nt in range(NT):
                        ps = sm_ps.tile([P, 512], F32, tag="sm")
                        for ck in range(CK):
                            nc.tensor.matmul(
                                ps[:],
                                wqkv_sb[:, ck, oc * 128 : (oc + 1) * 128],
                                x_t[:, ck, nt * 512 : (nt + 1) * 512],
                                start=(ck == 0),
                                stop=(ck == CK - 1),
                            )
                        nc.vector.tensor_scalar_add(
                            dst[:, oc % CK, nt * 512 : (nt + 1) * 512],
                            ps[:],
                            bqk_sb[:, oc : oc + 1],
                        )

                # V^T: token-major (t on partitions, v-channels free), augmented
                # with a ones column per head (65th column -> softmax denom)
                v_t = vp.tile([P, TK, NH * 65], BF16)
                v4 = v_t.rearrange("p tk (h e) -> p tk h e", e=65)
                nc.gpsimd.memset(v4[:, :, :, 64], 1.0)
                for tt in range(TK):
                    ps = sm_ps.tile([P, 512], F32, tag="sm")
                    for ck in range(CK):
                        nc.tensor.matmul(
                            ps[:],
                            x_t[:, ck, tt * 128 : (tt + 1) * 128],
                            wqkv_sb[:, ck, 2 * C : 3 * C],
                            start=(ck == 0),
                            stop=(ck == CK - 1),
                        )
                    nc.vector.tensor_copy(
                        v4[:, tt, :, 0:64],
                        ps.rearrange("p (h e) -> p h e", e=64),
                    )

                # attention per head
                attn_t = atp.tile([P, CK, T], BF16)
                for h in range(NH if mode != "qkvproj" else 0):
                    ec, eo = h // 2, (h % 2) * 64
                    pT = pp.tile([P, TK * T], BF16, tag="pT")  # flat (tk, t)
                    for tk in range(TK):
                        ps = qk_ps.tile([P, 1024], F32)
                        for nt in range(NT):
                            nc.tensor.matmul(
                                ps[:, nt * 512 : (nt + 1) * 512],
                                k_t[eo : eo + 64, ec, tk * 128 : (tk + 1) * 128],
                                q_t[eo : eo + 64, ec, nt * 512 : (nt + 1) * 512],
                                start=True,
                                stop=True,
                            )
                        nc.scalar.activation(
                            pT[:, tk * T : (tk + 1) * T],
                            ps[:],
                            EXP,
                            scale=SOFTMAX_SCALE,
                        )
                    for nt in range(NT):
                        avp = av_ps.tile([65, 512], F32)
                        for tk in range(TK):
                            nc.tensor.matmul(
                                avp[:],
                                v_t[:, tk, h * 65 : (h + 1) * 65],
                                pT[:, tk * T + nt * 512 : tk * T + (nt + 1) * 512],
                                start=(tk == 0),
                                stop=(tk == TK - 1),
                            )
                        if mode == "noepi":
                            nc.vector.tensor_copy(
                                attn_t[eo : eo + 64, ec, nt * 512 : (nt + 1) * 512],
                                avp[0:64, :],
                            )
                        else:
                            rec = sp.tile([1, 512], F32, tag="rec")
                            nc.vector.reciprocal(rec[:], avp[64:65, :])
                            rb = sp.tile([64, 512], F32, tag="rb")
                            nc.gpsimd.partition_broadcast(rb[:], rec[:])
                            nc.vector.tensor_mul(
                                attn_t[eo : eo + 64, ec, nt * 512 : (nt + 1) * 512],
                                avp[0:64, :],
                                rb[:],
                            )

                # output projection (o on partitions, t free) + bias, DMA out
                for ot in range(CK):
                    for nt in range(NT):
                        ps = sm_ps.tile([P, 512], F32, tag="sm")
                        for ck in range(CK):
                            nc.tensor.matmul(
                                ps[:],
                                wproj_sb[:, ck, ot * 128 : (ot + 1) * 128],
                                attn_t[:, ck, nt * 512 : (nt + 1) * 512],
                                start=(ck == 0),
                                stop=(ck == CK - 1),
                            )
                        y = sp.tile([P, 512], F32, tag="y")
                        nc.vector.tensor_scalar_add(
                            y[:], ps[:], bproj_sb[:, ot : ot + 1]
                        )
                        nc.sync.dma_start(
                            out_ext[
                                b, ot * 128 : (ot + 1) * 128, nt * 512 : (nt + 1) * 512
                            ],
                            y[:],
                        )
    nc.compile()
    return nc


def _get_nc():
    if not hasattr(_cache, "nc"):
        _cache.nc = _build_nc()
    return _cache.nc


def _prepare_in_maps(x, w_qkv, b_qkv, w_proj, b_proj):
    x = np.asarray(x, dtype=np.float32)
    w_qkv = np.asarray(w_qkv, dtype=np.float32)
    b_qkv = np.asarray(b_qkv, dtype=np.float32)
    w_proj = np.asarray(w_proj, dtype=np.float32)
    b_proj = np.asarray(b_proj, dtype=np.float32)

    bf16 = ml_dtypes.bfloat16
    wqkvT = np.ascontiguousarray(w_qkv.T).astype(bf16)          # (C, 3C)
    wprojT = np.ascontiguousarray(w_proj.T).astype(bf16)        # (C, C)
    # per-partition bias layouts: bias[j*128 + p] -> [p, j]
    bqk = np.ascontiguousarray(b_qkv[: 2 * C].reshape(2 * CK, P).T)
    # v-bias folds into the projection bias (softmax weights sum to 1)
    bproj_eff = w_proj @ b_qkv[2 * C :] + b_proj
    bproj = np.ascontiguousarray(bproj_eff.reshape(CK, P).T)

    xs = x.reshape(B, C, T).astype(bf16)
    in_maps = []
    for i in range(NCORES):
        in_maps.append(
            {
                "x": np.ascontiguousarray(xs[i * BLOC : (i + 1) * BLOC]),
                "wqkvT": wqkvT,
                "wprojT": wprojT,
                "bqk": bqk,
                "bproj": bproj,
            }
        )
    return in_maps


def kernel(x, w_qkv, b_qkv, w_proj, b_proj, _trace=False):
    from concourse.bass_utils import run_bass_kernel_spmd

    in_maps = _prepare_in_maps(x, w_qkv, b_qkv, w_proj, b_proj)
    nc = _get_nc()
    res = run_bass_kernel_spmd(
        nc, in_maps, core_ids=list(range(NCORES)), trace=_trace
    )
    out = np.concatenate([r["out"] for r in res.results], axis=0)
    out = out.reshape(B, C, 32, 32)
    if _trace:
        return out, res
    return out


if __name__ == "__main__":
    rng = np.random.default_rng(0)
    ins = {
        "x": rng.standard_normal((B, C, 32, 32), dtype=np.float32),
        "w_qkv": rng.standard_normal((3 * C, C), dtype=np.float32) / np.sqrt(C),
        "b_qkv": np.zeros(3 * C, np.float32),
        "w_proj": rng.standard_normal((C, C), dtype=np.float32) / np.sqrt(C),
        "b_proj": np.zeros(C, np.float32),
    }
    o = kernel(**ins)
    print("out", o.shape, o.dtype, float(np.abs(o).mean()))


# revision 9
# speedup vs baseline: 3.3107x; 2.4238x over previous
"""Trainium2 (8 NeuronCores) kernel for a 2D self-attention block.

Reference computation (per image, c=512 channels, t=h*w=1024 tokens, 8 heads):
    qkv  = w_qkv @ x + b_qkv           (1x1 conv == channel matmul)
    q,k,v split; per head: attn = softmax(q^T k / sqrt(64)); o = attn @ v
    out  = w_proj @ o + b_proj

Sharding: pure data-parallel — batch 16 split 2 images/core across 8 cores,
weights broadcast. No collectives needed.

Per-core dataflow (all matmul operands bf16, fp32 PSUM accumulation):
  - host pre-transposes weights -> wT (c-major contraction layouts on device)
  - Q,K computed channel-major (e,t); V computed token-major (t,e) so the
    attention matmuls need no on-chip transposes:
        scoresT = K_h^T Q_h   (T on partitions, t free)
        p = exp(scoresT/8)    (ScalarE, bf16 out; no max-subtraction needed:
                               logits are O(1) for this distribution)
        outT_aug = [V_h | 1]^T-chunks contracted with p  -> (65, t) where row
                   64 accumulates the softmax denominator (ones column trick)
        attn = outT_aug[0:64] * broadcast(1/row64)   (GpSimd partition bcast)
  - v-bias folds into an effective proj bias on host (softmax weights sum to 1)
  - proj: y^T-free layout (o on partitions, t free) -> direct DMA out
"""

import os
import sys
import threading

import numpy as np
import ml_dtypes

_REPO = "/opt/trn_rl_repo"
if _REPO not in sys.path:
    sys.path.insert(0, _REPO)

B, C, T = 16, 512, 1024
NH, E = 8, 64
NCORES = 8
BLOC = B // NCORES            # images per core
CK = C // 128                 # contraction chunks over channels
TK = T // 128                 # chunks over the T (attended) token axis
NT = T // 512                 # 512-wide tiles over the t axis
P = 128
SOFTMAX_SCALE = 1.0 / 8.0     # 1/sqrt(E)

_cache = threading.local()


def _build_nc(reps=1, mode="full"):
    import concourse.tile as tile
    from concourse import bacc, mybir

    F32 = mybir.dt.float32
    BF16 = mybir.dt.bfloat16
    EXP = mybir.ActivationFunctionType.Exp

    nc = bacc.Bacc(None, target_bir_lowering=False, debug=False)
    x_ext = nc.declare_dram_parameter("x", [BLOC, C, T], BF16, isOutput=False)
    wqkv_ext = nc.declare_dram_parameter("wqkvT", [C, 3 * C], BF16, isOutput=False)
    wproj_ext = nc.declare_dram_parameter("wprojT", [C, C], BF16, isOutput=False)
    bqk_ext = nc.declare_dram_parameter("bqk", [P, 8], F32, isOutput=False)
    bproj_ext = nc.declare_dram_parameter("bproj", [P, CK], F32, isOutput=False)
    out_ext = nc.declare_dram_parameter("out", [BLOC, C, T], F32, isOutput=True)

    with tile.TileContext(nc) as tc:
        with (
            tc.tile_pool(name="consts", bufs=1) as consts,
            tc.tile_pool(name="xp", bufs=2) as xp,
            tc.tile_pool(name="qkp", bufs=2) as qkp,
            tc.tile_pool(name="vp", bufs=2) as vp,
            tc.tile_pool(name="pp", bufs=3) as pp,
            tc.tile_pool(name="atp", bufs=2) as atp,
            tc.tile_pool(name="sp", bufs=4) as sp,
            tc.tile_pool(name="qk_ps", bufs=2, space="PSUM") as qk_ps,
            tc.tile_pool(name="av_ps", bufs=2, space="PSUM") as av_ps,
            tc.tile_pool(name="sm_ps", bufs=2, space="PSUM") as sm_ps,
        ):
            wqkv_sb = consts.tile([P, CK, 3 * C], BF16)
            nc.sync.dma_start(
                wqkv_sb[:], wqkv_ext.rearrange("(ck p) o -> p ck o", p=P)
            )
            wproj_sb = consts.tile([P, CK, C], BF16)
            nc.sync.dma_start(
                wproj_sb[:], wproj_ext.rearrange("(ck p) o -> p ck o", p=P)
            )
            bqk_sb = consts.tile([P, 8], F32)
            nc.sync.dma_start(bqk_sb[:], bqk_ext[:])
            bproj_sb = consts.tile([P, CK], F32)
            nc.sync.dma_start(bproj_sb[:], bproj_ext[:])

            for b in [b for _ in range(reps) for b in range(BLOC)]:
                x_t = xp.tile([P, CK, T], BF16)
                nc.sync.dma_start(
                    x_t[:], x_ext[b].rearrange("(ck p) t -> p ck t", p=P)
                )

                # Q, K projections: channel-major (e on partitions, t free)
                q_t = qkp.tile([P, CK, T], BF16, tag="q")
                k_t = qkp.tile([P, CK, T], BF16, tag="k")
                for oc in range(2 * CK):
                    dst = q_t if oc < CK else k_t
                    for nt in range(NT):
                        ps = sm_ps.tile([P, 512], F32, tag="sm")
                        for ck in range(CK):
                            nc.tensor.matmul(
                                ps[:],
                                wqkv_sb[:, ck, oc * 128 : (oc + 1) * 128],
                                x_t[:, ck, nt * 512 : (nt + 1) * 512],
                                start=(ck == 0),
                                stop=(ck == CK - 1),
                            )
                        nc.vector.tensor_scalar_add(
                            dst[:, oc % CK, nt * 512 : (nt + 1) * 512],
                            ps[:],
                            bqk_sb[:, oc : oc + 1],
                        )

                # V^T: token-major (t on partitions, v-channels free), augmented
                # with a ones column per head (65th column -> softmax denom)
                v_t = vp.tile([P, TK, NH * 65], BF16)
                v4 = v_t.rearrange("p tk (h e) -> p tk h e", e=65)
                nc.gpsimd.memset(v4[:, :, :, 64], 1.0)
                for tt in range(TK):
                    ps = sm_ps.tile([P, 512], F32, tag="sm")
                    for ck in range(CK):
                        nc.tensor.matmul(
                            ps[:],
                            x_t[:, ck, tt * 128 : (tt + 1) * 128],
                            wqkv_sb[:, ck, 2 * C : 3 * C],
                            start=(ck == 0),
                            stop=(ck == CK - 1),
                        )
                    nc.vector.tensor_copy(
                        v4[:, tt, :, 0:64],
                        ps.rearrange("p (h e) -> p h e", e=64),
                    )

                # attention per head
                attn_t = atp.tile([P, CK, T], BF16)
                if mode == "qkvproj":
                    nc.vector.memset(attn_t[:], 1.0)
                for h in range(NH if mode != "qkvproj" else 0):
                    ec, eo = h // 2, (h % 2) * 64
                    pT = pp.tile([P, TK * T], BF16, tag="pT")  # flat (tk, t)
                    for tk in range(TK):
                        ps = qk_ps.tile([P, 1024], F32)
                        for nt in range(NT):
                            nc.tensor.matmul(
                                ps[:, nt * 512 : (nt + 1) * 512],
                                k_t[eo : eo + 64, ec, tk * 128 : (tk + 1) * 128],
                                q_t[eo : eo + 64, ec, nt * 512 : (nt + 1) * 512],
                                start=True,
                                stop=True,
                            )
                        nc.scalar.activation(
                            pT[:, tk * T : (tk + 1) * T],
                            ps[:],
                            EXP,
                            scale=SOFTMAX_SCALE,
                        )
                    for nt in range(NT):
                        avp = av_ps.tile([65, 512], F32)
                        for tk in range(TK):
                            nc.tensor.matmul(
                                avp[:],
                                v_t[:, tk, h * 65 : (h + 1) * 65],
                                pT[:, tk * T + nt * 512 : tk * T + (nt + 1) * 512],
                                start=(tk == 0),
                                stop=(tk == TK - 1),
                            )
                        if mode == "noepi":
                            nc.vector.tensor_copy(
                                attn_t[eo : eo + 64, ec, nt * 512 : (nt + 1) * 512],
                                avp[0:64, :],
                            )
                        else:
                            rec = sp.tile([1, 512], F32, tag="rec")
                            nc.vector.reciprocal(rec[:], avp[64:65, :])
                            rb = sp.tile([64, 512], F32, tag="rb")
                            nc.gpsimd.partition_broadcast(rb[:], rec[:])
                            nc.vector.tensor_mul(
                                attn_t[eo : eo + 64, ec, nt * 512 : (nt + 1) * 512],
                                avp[0:64, :],
                                rb[:],
                            )

                # output projection (o on partitions, t free) + bias, DMA out
                for ot in range(CK):
                    for nt in range(NT):
                        ps = sm_ps.tile([P, 512], F32, tag="sm")
                        for ck in range(CK):
                            nc.tensor.matmul(
                                ps[:],
                                wproj_sb[:, ck, ot * 128 : (ot + 1) * 128],
                                attn_t[:, ck, nt * 512 : (nt + 1) * 512],
                                start=(ck == 0),
                                stop=(ck == CK - 1),
                            )
                        y = sp.tile([P, 512], F32, tag="y")
                        nc.vector.tensor_scalar_add(
                            y[:], ps[:], bproj_sb[:, ot : ot + 1]
                        )
                        nc.sync.dma_start(
                            out_ext[
                                b, ot * 128 : (ot + 1) * 128, nt * 512 : (nt + 1) * 512
                            ],
                            y[:],
                        )
    nc.compile()
    return nc


def _get_nc():
    if not hasattr(_cache, "nc"):
        _cache.nc = _build_nc()
    return _cache.nc


def _prepare_in_maps(x, w_qkv, b_qkv, w_proj, b_proj):
    x = np.asarray(x, dtype=np.float32)
    w_qkv = np.asarray(w_qkv, dtype=np.float32)
    b_qkv = np.asarray(b_qkv, dtype=np.float32)
    w_proj = np.asarray(w_proj, dtype=np.float32)
    b_proj = np.asarray(b_proj, dtype=np.float32)

    bf16 = ml_dtypes.bfloat16
    wqkvT = np.ascontiguousarray(w_qkv.T).astype(bf16)          # (C, 3C)
    wprojT = np.ascontiguousarray(w_proj.T).astype(bf16)        # (C, C)
    # per-partition bias layouts: bias[j*128 + p] -> [p, j]
    bqk = np.ascontiguousarray(b_qkv[: 2 * C].reshape(2 * CK, P).T)
    # v-bias folds into the projection bias (softmax weights sum to 1)
    bproj_eff = w_proj @ b_qkv[2 * C :] + b_proj
    bproj = np.ascontiguousarray(bproj_eff.reshape(CK, P).T)

    xs = x.reshape(B, C, T).astype(bf16)
    in_maps = []
    for i in range(NCORES):
        in_maps.append(
            {
                "x": np.ascontiguousarray(xs[i * BLOC : (i + 1) * BLOC]),
                "wqkvT": wqkvT,
                "wprojT": wprojT,
                "bqk": bqk,
                "bproj": bproj,
            }
        )
    return in_maps


def kernel(x, w_qkv, b_qkv, w_proj, b_proj, _trace=False):
    from concourse.bass_utils import run_bass_kernel_spmd

    in_maps = _prepare_in_maps(x, w_qkv, b_qkv, w_proj, b_proj)
    nc = _get_nc()
    res = run_bass_kernel_spmd(
        nc, in_maps, core_ids=list(range(NCORES)), trace=_trace
    )
    out = np.concatenate([r["out"] for r in res.results], axis=0)
    out = out.reshape(B, C, 32, 32)
    if _trace:
        return out, res
    return out


if __name__ == "__main__":
    rng = np.random.default_rng(0)
    ins = {
        "x": rng.standard_normal((B, C, 32, 32), dtype=np.float32),
        "w_qkv": rng.standard_normal((3 * C, C), dtype=np.float32) / np.sqrt(C),
        "b_qkv": np.zeros(3 * C, np.float32),
        "w_proj": rng.standard_normal((C, C), dtype=np.float32) / np.sqrt(C),
        "b_proj": np.zeros(C, np.float32),
    }
    o = kernel(**ins)
    print("out", o.shape, o.dtype, float(np.abs(o).mean()))
